# revision 43
# baseline (speedup 1.0000x reference)
"""Trainium2 Bass kernel for nn_CCNN2d (convolutional coupled NN 2d).

Per-sample recurrence (T steps), per core (pure data parallel over B=8):
    f = DF*f + conv3x3(y, conv_w) + bias + x_t
    l = DL*l + conv3x3(y, wf)          # wf[o,i,:,:] = k  (same for all o,i)
    u = f * (1 + 0.5*l)
    e = DE*e + VE*y      (tracked as ehat = e/VE;  ehat = DE*ehat + y)
    y = sigmoid(u - e)   (= sigmoid(u - VE*ehat))

Layout (per core): C=32 channels, H=128 rows in G=4 bands of 32 rows.
SBUF partition 32*g + c  <-  (band g, channel c).
Conv as matmul on 32x32 PE subarray tiles:
  cf chain (row g, col g+rot): 27 split-conv taps + identity*(x+bias) taps.
  lc chain (row g, col g+rot+1): 8 taps of k[dy,dx]*ones (center k=0).
    -> L state kept in "rotated by rot+1" layout.
  T1 = 1 + 0.5*L is rotated back to rot via identity matmuls (pT chain).

Per-chunk partition rotation: chunk state (F/L/E/flat-y/psum) for chunks
{2,3} is stored rotated +2 partition groups vs chunks {0,1}.  Elementwise
ops are layout-agnostic; conv-input y planes stay natural.  Consecutive
chunks in processing order (1,2,0,3) then use DISJOINT 32x32 PE tilesets
(col rotations 0/1 vs 2/3), so two chunks' matmul chains stream through
the PE array concurrently (up to 16 active tiles instead of 8).
Order (1,2,0,3) also lets step t+1's first chunk start before step t's
last chunk finishes its elementwise tail (keeps PE warm, HAM at 8/8).
"""

import numpy as np

import concourse.bass as bass
import concourse.mybir as mybir
import concourse.tile as tile
from concourse.bass_utils import run_bass_kernel_spmd


F32 = mybir.dt.float32
F32R = mybir.dt.float32r
BF16 = mybir.dt.bfloat16
ALU = mybir.AluOpType
ACTF = mybir.ActivationFunctionType

ALPHA_F, ALPHA_L, ALPHA_E, V_E = 0.1, 1.0, 1.0, 10.0
DF = float(np.exp(-ALPHA_F))
DL = float(np.exp(-ALPHA_L))
DE = float(np.exp(-ALPHA_E))

B, T, C, H, W = 8, 10, 32, 128, 128
G = 4  # partition groups / row bands
KVALS = [[0.5, 1.0, 0.5], [1.0, 0.0, 1.0], [0.5, 1.0, 0.5]]

PAIRS = [(0, 3), (1, 2)]
# chunk -> partition-group rotation of its state/psum.  The two chunks of
# a pair use rotations {0, 2}, giving disjoint PE diagonals (8 concurrent
# 32x32 tiles) while keeping un-rotation partition windows 64-aligned
# (the cheap 2-segment engine-copy case).
ROT = {0: 0, 3: 2, 1: 0, 2: 2}


def build_nc(t_steps=T, h=H, w=W):
    br = h // G          # rows per band
    pw = w + 2           # padded width
    ph = br + 2          # padded rows per band
    n_chunks = br // 8   # 8 image rows (2 psum banks) per chunk
    assert br % 8 == 0 and w == 128

    nc = bass.Bass()
    x_ext = nc.declare_dram_parameter("x", [t_steps, C, h, w], F32, isOutput=False)
    w_ext = nc.declare_dram_parameter("conv_w", [C, C, 3, 3], F32, isOutput=False)
    b_ext = nc.declare_dram_parameter("conv_b", [C], F32, isOutput=False)
    y_ext = nc.declare_dram_parameter("y", [t_steps, C, h, w], F32, isOutput=True)

    from contextlib import ExitStack
    with tile.TileContext(nc) as tc, ExitStack() as ctx:
        _build(ctx, tc, nc, x_ext, w_ext, b_ext, y_ext, t_steps, h, w, br, ph, pw, n_chunks)
    return nc


def _split_matmul_waits(nc):
    """walrus's S3_LW (matmul weight-load) struct has a single sync-wait
    slot; Tile sometimes attaches 2+. Move all-but-one wait onto an
    inserted PE EventSemaphore right before the matmul."""
    import copy as _copy

    split_types = {
        "InstMatmult", "InstTensorTensor", "InstTensorScalarPtr",
        "InstActivation", "InstTensorCopy", "InstStreamTranspose",
        "InstMemset", "InstTensorScalarAffineSelect", "InstTensorReduce",
        "InstDMACopy", "InstTensorLoad", "InstTensorSave", "InstDrain", "InstNoOp",
    }
    fn = nc.m.functions[0]
    new_blocks = []
    for bb in fn.blocks:
        out = []
        changed = False
        for ins in bb.instructions:
            si = ins.sync_info
            if (type(ins).__name__ in split_types and si is not None
                    and si.on_wait and len(si.on_wait) > 1):
                waits = list(si.on_wait)
                for i in range(0, len(waits), 2):  # <=2 waits per EventSemaphore
                    out.append(mybir.InstEventSemaphore(
                        name=nc.get_next_instruction_name(),
                        engine=ins.engine,
                        ins=[], outs=[],
                        sync_info=mybir.SyncInfo(
                            on_wait=waits[i:i + 2], on_update=[]),
                    ))
                ins.sync_info = mybir.SyncInfo(
                    on_wait=[], on_update=list(si.on_update or []))
                changed = True
            out.append(ins)
        if changed:
            new_blocks.append(_copy.replace(bb, instructions=out))
        else:
            new_blocks.append(bb)
    new_fn = _copy.replace(fn, blocks=[])
    new_fn.set_allocations_from_list(fn.allocations)
    new_fn.blocks.extend(new_blocks)
    nc.m = _copy.replace(nc.m, functions=[])
    nc.m.functions.append(new_fn)


def _build(ctx, tc, nc, x_ext, w_ext, b_ext, y_ext, t_steps, h, w, br, ph, pw, n_chunks):
    singles = ctx.enter_context(tc.tile_pool(name="singles", bufs=1))
    xbpool = ctx.enter_context(tc.tile_pool(name="xbpool", bufs=2))
    tmp_pool = ctx.enter_context(tc.tile_pool(name="tmps", bufs=2))
    psA_pool = ctx.enter_context(tc.tile_pool(name="psA", bufs=2, space="PSUM"))
    psB_pool = ctx.enter_context(tc.tile_pool(name="psB", bufs=2, space="PSUM"))

    # ---- persistent state ----
    Yf0 = singles.tile([128, br * w], F32, tag="Yf0")   # flat fp32 y (ping)
    Yf1 = singles.tile([128, br * w], F32, tag="Yf1")   # flat fp32 y (pong)
    Yh0 = singles.tile([128, ph * pw], BF16, tag="Yh0")  # padded bf16 hi
    Yl0 = singles.tile([128, ph * pw], BF16, tag="Yl0")  # padded bf16 lo
    Yh1 = singles.tile([128, ph * pw], BF16, tag="Yh1")
    Yl1 = singles.tile([128, ph * pw], BF16, tag="Yl1")
    F = singles.tile([128, br * w], F32, tag="F")
    M = singles.tile([128, br * w], F32, tag="M")   # M = 1+0.5*l, same rot as F
    E = singles.tile([128, br * w], F32, tag="E")   # ehat = e / V_E

    # ---- constants ----
    Wcf = singles.tile([128, 9 * 32], F32, tag="Wcf")    # fp32 staging, replicated
    Wch = singles.tile([128, 9 * 32], BF16, tag="Wch")   # bf16 hi
    Wcl = singles.tile([128, 9 * 32], BF16, tag="Wcl")   # bf16 lo
    Wstage = singles.tile([32, 9 * 32], F32, tag="Wstage")
    IDT = singles.tile([128, 32], BF16, tag="IDT")
    IDF = singles.tile([128, 32], F32, tag="IDF")        # fp32 identity
    K05 = singles.tile([128, 32], BF16, tag="K05")      # 0.25*ones (lc taps k=0.5, halved)
    K10 = singles.tile([128, 32], BF16, tag="K10")      # 0.5*ones  (lc taps k=1.0, halved)
    ONESF = singles.tile([128, 512], F32, tag="ONESF")   # fp32 ones rhs
    CONF = singles.tile([128, 32], F32, tag="CONF")      # (1-DL)/32 fp32

    for yy in (Yh0, Yl0, Yh1, Yl1):
        nc.vector.memset(yy, 0.0)
    nc.vector.memset(Yf0, 0.0)
    nc.vector.memset(Yf1, 0.0)
    nc.vector.memset(F, 0.0)
    nc.vector.memset(M, 1.0)
    # e0 = V_E/ALPHA_E, ehat0 = e0/V_E = 1/ALPHA_E
    nc.vector.memset(E, 1.0 / ALPHA_E)
    nc.vector.memset(K05, 0.25)
    nc.vector.memset(K10, 0.5)
    nc.vector.memset(ONESF, 1.0)
    nc.vector.memset(CONF, (1.0 - DL) / 32.0)

    # conv weights: stage as [o, (i ky kx)] contiguous, transpose per tap to [i, o]
    nc.sync.dma_start(out=Wstage, in_=w_ext[:].rearrange("o i ky kx -> o (i ky kx)"))
    wst = Wstage.rearrange("p (i t) -> p i t", t=9)
    for tap in range(9):
        nc.vector.transpose(out=Wcf[0:32, 32 * tap:32 * tap + 32], in_=wst[:, :, tap])
    # identity: gpsimd can't touch f32r, so build in an f32 staging tile
    # and cast-copy into IDT on DVE (exact for 0/1 values)
    IDTs = singles.tile([32, 32], F32, tag="IDTs")
    nc.gpsimd.memset(IDTs, 0.0)
    nc.gpsimd.affine_select(
        out=IDTs, in_=IDTs,
        compare_op=ALU.not_equal, fill=1.0, base=0,
        pattern=[[-1, 32]], channel_multiplier=1)
    nc.vector.tensor_copy(out=IDT[0:32, :], in_=IDTs)
    nc.vector.tensor_copy(out=IDF[0:32, :], in_=IDTs)
    # replicate fp32 weights to groups 1..3, then hi/lo bf16 split
    for g in range(1, G):
        nc.vector.tensor_copy(out=Wcf[32 * g:32 * g + 32, :], in_=Wcf[0:32, :])
        nc.vector.tensor_copy(out=IDT[32 * g:32 * g + 32, :], in_=IDT[0:32, :])
        nc.vector.tensor_copy(out=IDF[32 * g:32 * g + 32, :], in_=IDF[0:32, :])
    nc.vector.tensor_copy(out=Wch, in_=Wcf)
    nc.vector.scalar_tensor_tensor(out=Wcl, in0=Wcf, scalar=1.0, in1=Wch,
                                   op0=ALU.mult, op1=ALU.subtract)

    taps = [(ky, kx) for ky in range(3) for kx in range(3)]
    lc_taps = [(ky, kx) for (ky, kx) in taps if KVALS[ky][kx] != 0.0]

    def s2_compute_half(c, lb):
        """u/e/sigmoid for one lb half ([128,512]): yf_out[:, slh] ready."""
        t, ch, rot = c["t"], c["ch"], c["rot"]
        yf_in, yf_out = c["yf_in"], c["yf_out"]
        co = 512 * lb
        slh = slice(1024 * ch + co, 1024 * ch + co + 512)
        if "U" not in c:
            U = tmp_pool.tile([128, 1024], F32, tag="U")
            c["U"] = U
        Uh = c["U"][:, co:co + 512]
        nc.vector.tensor_tensor(out=Uh, in0=F[:, slh], in1=M[:, slh],
                                op=ALU.mult)
        # ehat = DE*ehat + y_old (flat fp32 y)
        nc.vector.scalar_tensor_tensor(
            out=E[:, slh], in0=E[:, slh], scalar=DE, in1=yf_in[:, slh],
            op0=ALU.mult, op1=ALU.add)
        nc.vector.scalar_tensor_tensor(
            out=Uh, in0=E[:, slh], scalar=-V_E, in1=Uh,
            op0=ALU.mult, op1=ALU.add)
        nc.scalar.activation(out=yf_out[:, slh], in_=Uh, func=ACTF.Sigmoid)

    def s2_planes_half(c, lb):
        """y store + bf16 hi/lo conv-plane build for one lb half.  Rotated
        chunks (rot == 2) split hi/lo in rotated layout first, then
        un-rotate with 1-input engine copies in the two 64-aligned
        partition windows (legal window must start at 0/32/64/96 and stay
        inside its alignment block).  The HBM y store goes on the scalar
        engine's HW DGE ring -- nothing in-kernel reads y_ext."""
        t, ch, rot = c["t"], c["ch"], c["rot"]
        yf_out = c["yf_out"]
        yhov, ylov = c["yhov"], c["ylov"]
        co = 512 * lb
        slh = slice(1024 * ch + co, 1024 * ch + co + 512)
        r0 = 8 * ch + 4 * lb
        y_dst = y_ext[t].rearrange("c (g r) w -> g c r w", g=G)
        # plane builds FIRST (critical path: next step's convs), y store
        # DMAs after -- on the otherwise-idle sync ring so their issue
        # never queues ahead of latency-critical scalar/vector ops
        if rot == 0:
            src = yf_out[:, slh].rearrange("p (r c) -> p r c", c=w)
            if t + 1 < t_steps:
                yho_int = yhov[:, r0 + 1:r0 + 5, 1:1 + w]
                ylo_int = ylov[:, r0 + 1:r0 + 5, 1:1 + w]
                nc.scalar.copy(out=yho_int, in_=src)
                nc.vector.tensor_tensor(out=ylo_int, in0=src, in1=yho_int,
                                        op=ALU.subtract)
            nc.sync.dma_start(out=y_dst[:, :, r0:r0 + 4, :], in_=src)
        else:
            if t + 1 < t_steps:
                Yhr = tmp_pool.tile([128, 512], BF16, tag="Yhr")
                Ylr = tmp_pool.tile([128, 512], BF16, tag="Ylr")
                nc.scalar.copy(out=Yhr, in_=yf_out[:, slh])
                nc.vector.tensor_tensor(out=Ylr, in0=yf_out[:, slh], in1=Yhr,
                                        op=ALU.subtract)
                for (p0, npt) in ((0, 64), (64, 64)):
                    s0 = (p0 + 32 * rot) % 128
                    nc.scalar.copy(
                        out=yhov[p0:p0 + npt, r0 + 1:r0 + 5, 1:1 + w],
                        in_=Yhr[s0:s0 + npt, :].rearrange(
                            "p (r c) -> p r c", c=w))
                    nc.vector.tensor_copy(
                        out=ylov[p0:p0 + npt, r0 + 1:r0 + 5, 1:1 + w],
                        in_=Ylr[s0:s0 + npt, :].rearrange(
                            "p (r c) -> p r c", c=w))
            for g in range(G):
                g2 = (g + rot) % G
                nc.sync.dma_start(
                    out=y_dst[g, :, r0:r0 + 4, :],
                    in_=yf_out[32 * g2:32 * g2 + 32, slh])
        # halo rows for next step's convs, as soon as the producing rows
        # exist: chunk 0 lb0 (row 1) -> bottom halos (row br+1); chunk 3
        # lb1 (row br) -> top halos (row 0).  yh halos on scalar (right
        # behind sigmoid/yho), yl halos on vector.
        if ch == 0 and lb == 0:
            for g in range(G - 1):
                nc.scalar.copy(out=yhov[32 * g:32 * g + 32, br + 1, :],
                               in_=yhov[32 * (g + 1):32 * (g + 2), 1, :])
            for g in range(G - 1):
                nc.vector.tensor_copy(out=ylov[32 * g:32 * g + 32, br + 1, :],
                                      in_=ylov[32 * (g + 1):32 * (g + 2), 1, :])
        if ch == 3 and lb == 1:
            for g in range(1, G):
                nc.scalar.copy(out=yhov[32 * g:32 * g + 32, 0, :],
                               in_=yhov[32 * (g - 1):32 * g, br, :])
            for g in range(1, G):
                nc.vector.tensor_copy(out=ylov[32 * g:32 * g + 32, 0, :],
                                      in_=ylov[32 * (g - 1):32 * g, br, :])

    def emit_stage2(c):
        """Post-PSUM tail of a chunk: u/e, sigmoid, y store, plane builds,
        halo copies -- pure DVE/ACT/DMA work, no PE involvement."""
        for lb in range(2):
            s2_compute_half(c, lb)
        for lb in range(2):
            s2_planes_half(c, lb)

    prev = None
    deferred_m = []
    for t in range(t_steps):
        yf_in = Yf0 if t % 2 == 0 else Yf1
        yf_out = Yf1 if t % 2 == 0 else Yf0
        yh_in, yl_in = (Yh0, Yl0) if t % 2 == 0 else (Yh1, Yl1)
        yh_out, yl_out = (Yh1, Yl1) if t % 2 == 0 else (Yh0, Yl0)
        yhv = yh_in.rearrange("p (r c) -> p r c", c=pw)
        ylv = yl_in.rearrange("p (r c) -> p r c", c=pw)
        yhov = yh_out.rearrange("p (r c) -> p r c", c=pw)
        ylov = yl_out.rearrange("p (r c) -> p r c", c=pw)

        xb = xbpool.tile([128, br * w], F32, tag="xb")
        # x load on the scalar HW DGE ring: the sync ring carries the y
        # stores, whose backlog must not delay next-step x availability
        nc.scalar.dma_start(out=xb, in_=x_ext[t].rearrange("c (g r) w -> g c r w", g=G))

        def cf_lb(ch, lb, pA):
            rot = ROT[ch]
            gb = 2 * ch + lb
            co = 512 * lb
            tlist = taps
            if ch == 3 and lb == 0:
                # plane row 8*ch (= ky=0 rows here) is chunk 2's last
                # row, whose stage2 flushes last at the step boundary:
                # emit those taps last so the chain head never stalls
                tlist = [(ky, kx) for ky in (1, 2, 0) for kx in range(3)]
            for ti0, (ky, kx) in enumerate(tlist):
                ti = taps.index((ky, kx))
                for (wt, yv) in ((Wch, yhv), (Wch, ylv), (Wcl, yhv)):
                    def emit(ti0=ti0, ti=ti, ky=ky, kx=kx, wt=wt, yv=yv,
                             gb=gb, co=co, rot=rot, pA=pA):
                        for g in range(G):
                            g2 = (g + rot) % G
                            nc.tensor.matmul(
                                out=pA[32 * g2:32 * g2 + 32, co:co + 512],
                                lhsT=wt[32 * g:32 * g + 32,
                                        32 * ti:32 * ti + 32],
                                rhs=yv[32 * g:32 * g + 32,
                                       4 * gb + ky:4 * gb + ky + 4,
                                       kx:kx + 128],
                                start=(ti0 == 0 and wt is Wch and yv is yhv),
                                stop=False, skip_group_check=True,
                                tile_position=(32 * g, 32 * g2),
                            )
                    yield emit
            def emit_x(gb=gb, co=co, rot=rot, pA=pA):
                for g in range(G):
                    g2 = (g + rot) % G
                    nc.tensor.matmul(
                        out=pA[32 * g2:32 * g2 + 32, co:co + 512],
                        lhsT=IDF[32 * g:32 * g + 32, 0:32],
                        rhs=xb[32 * g:32 * g + 32, 512 * gb:512 * gb + 512],
                        start=False, stop=True, skip_group_check=True,
                        tile_position=(32 * g, 32 * g2),
                    )
            yield emit_x

        def lc_lb(ch, lb, pB):
            rot = ROT[ch]
            gb = 2 * ch + lb
            co = 512 * lb
            for ti, (ky, kx) in enumerate(lc_taps):
                kt = K05 if KVALS[ky][kx] == 0.5 else K10
                for yi, yv in enumerate((yhv, ylv)):
                    def emit(ti=ti, ky=ky, kx=kx, kt=kt, yv=yv, yi=yi,
                             gb=gb, co=co, rot=rot, pB=pB):
                        for g in range(G):
                            g3 = (g + rot) % G
                            nc.tensor.matmul(
                                out=pB[32 * g3:32 * g3 + 32, co:co + 512],
                                lhsT=kt[32 * g:32 * g + 32, 0:32],
                                rhs=yv[32 * g:32 * g + 32,
                                       4 * gb + ky:4 * gb + ky + 4,
                                       kx:kx + 128],
                                start=(ti == 0 and yi == 0),
                                stop=False, skip_group_check=True,
                                tile_position=(32 * g, 32 * g3),
                            )
                    yield emit
            def emit_c(co=co, rot=rot, pB=pB):
                for g in range(G):
                    g3 = (g + rot) % G
                    nc.tensor.matmul(
                        out=pB[32 * g3:32 * g3 + 32, co:co + 512],
                        lhsT=CONF[32 * g:32 * g + 32, 0:32],
                        rhs=ONESF[32 * g:32 * g + 32, 0:512],
                        start=False, stop=True, skip_group_check=True,
                        tile_position=(32 * g, 32 * g3),
                    )
            yield emit_c

        def run_rr(chains):
            while chains:
                nxt = []
                for it in chains:
                    e = next(it, None)
                    if e is not None:
                        e()
                        nxt.append(it)
                chains = nxt

        def f_stt(ch, pA):
            sl = slice(1024 * ch, 1024 * ch + 1024)
            nc.vector.scalar_tensor_tensor(
                out=F[:, sl], in0=F[:, sl], scalar=DF, in1=pA[:, :],
                op0=ALU.mult, op1=ALU.add)

        def m_stt(ch, pB):
            sl = slice(1024 * ch, 1024 * ch + 1024)
            nc.vector.scalar_tensor_tensor(
                out=M[:, sl], in0=M[:, sl], scalar=DL, in1=pB[:, :],
                op0=ALU.mult, op1=ALU.add)

        def m_stt_h(ch, pB, lb):
            slh = slice(1024 * ch + 512 * lb, 1024 * ch + 512 * lb + 512)
            nc.vector.scalar_tensor_tensor(
                out=M[:, slh], in0=M[:, slh], scalar=DL,
                in1=pB[:, 512 * lb:512 * lb + 512],
                op0=ALU.mult, op1=ALU.add)

        def s2ctx(ch):
            return {"t": t, "ch": ch, "rot": ROT[ch],
                    "sl": slice(1024 * ch, 1024 * ch + 1024),
                    "yf_in": yf_in, "yf_out": yf_out,
                    "yhov": yhov, "ylov": ylov}

        # Each pair's four lb half-chains use all four PE diagonals -> 16
        # concurrent 32x32 tiles while only two chunks' psum is in flight.
        # stage2 (pure DVE/ACT/DMA) is deferred one pair-slot -- INCLUDING
        # across the step boundary -- so its elementwise work always
        # overlaps the NEXT pair's matmul streams.  The boundary-flushed
        # pair (1, 2) runs HALF-chunk granular in criticality order: the
        # next step's earliest stalled taps read plane row 9 (ch1 lb0) and
        # row 24 (ch2 lb1), so those halves complete first.
        for pi, (ca, cb) in enumerate(PAIRS):
            if prev is not None:
                if prev[0]["ch"] == PAIRS[1][0]:
                    c1, c2 = prev
                    for (c, lb) in ((c1, 0), (c2, 1), (c1, 1), (c2, 0)):
                        s2_compute_half(c, lb)
                        s2_planes_half(c, lb)
                else:
                    for c in prev:
                        emit_stage2(c)
            prev = [s2ctx(ca), s2ctx(cb)]
            pAa = psA_pool.tile([128, 1024], F32, tag="pA")
            pAb = psA_pool.tile([128, 1024], F32, tag="pA")
            run_rr([cf_lb(ca, 0, pAa), cf_lb(ca, 1, pAa),
                    cf_lb(cb, 0, pAb), cf_lb(cb, 1, pAb)])
            f_stt(ca, pAa)
            f_stt(cb, pAb)
            pBa = psB_pool.tile([128, 1024], F32, tag="pBT")
            pBb = psB_pool.tile([128, 1024], F32, tag="pBT")
            run_rr([lc_lb(ca, 0, pBa), lc_lb(ca, 1, pBa),
                    lc_lb(cb, 0, pBb), lc_lb(cb, 1, pBb)])
            if pi == 1:
                # boundary pair: half-granular M updates, critical halves
                # first, so the flush's dependent chains start earliest
                m_stt_h(ca, pBa, 0)
                m_stt_h(cb, pBb, 1)
                m_stt_h(ca, pBa, 1)
                m_stt_h(cb, pBb, 0)
            else:
                m_stt(ca, pBa)
                m_stt(cb, pBb)
    # final flush: last pair's stage2 (y store for the last step)
    c1, c2 = prev
    for (c, lb) in ((c1, 0), (c2, 1), (c1, 1), (c2, 0)):
        s2_compute_half(c, lb)
        s2_planes_half(c, lb)


def _thin_matmul_incs(nc):
    """Every matmul increments the PE counting semaphore at @complete; the
    EVT_SEM register write serializes (~26ns each), rate-limiting MM issue
    to ~34ns even when streams could overlap.  Only ~1% of those counts are
    ever waited on.  Keep the increment only on MMs whose cumulative index
    appears as a wait threshold (MMs complete in pc order, so the t-th inc
    completing implies MMs 1..t are done) and renumber all wait thresholds
    to ranks within the kept set."""
    fns = list(nc.m.functions)
    # identify the semaphore MMs increment
    mm_sem = None
    for fn in fns:
        for bb in fn.blocks:
            for ins in bb.instructions:
                if type(ins).__name__ == "InstMatmult" and ins.sync_info:
                    for u in ins.sync_info.on_update:
                        if u.update_mode == "sem-inc":
                            mm_sem = u.id
                            break
                if mm_sem is not None:
                    break
            if mm_sem is not None:
                break
        if mm_sem is not None:
            break
    if mm_sem is None:
        return
    # collect wait thresholds on mm_sem (all immediate)
    thresholds = set()
    n_mm = 0
    for fn in fns:
        for bb in fn.blocks:
            for ins in bb.instructions:
                if type(ins).__name__ == "InstMatmult":
                    n_mm += 1
                si = ins.sync_info
                if not si:
                    continue
                for w in si.on_wait:
                    if w.id == mm_sem and w.sync_type == "semaphore":
                        assert w.wait_mode == "sem-ge-imm" and w.wait_reg is None, \
                            (w.wait_mode, w.wait_reg)
                        thresholds.add(w.wait_value)
    keep = sorted(thresholds | {n_mm})
    rank = {t: i + 1 for i, t in enumerate(keep)}
    keepset = set(keep)
    # rewrite
    c = 0
    for fn in fns:
        for bb in fn.blocks:
            for ins in bb.instructions:
                si = ins.sync_info
                is_mm = type(ins).__name__ == "InstMatmult"
                if is_mm:
                    c += 1
                if not si:
                    continue
                new_wait = []
                changed = False
                for w in si.on_wait:
                    if w.id == mm_sem and w.sync_type == "semaphore":
                        new_wait.append(mybir.SyncWait(
                            sync_type=w.sync_type, id=w.id,
                            wait_mode=w.wait_mode, ant_name=w.ant_name,
                            wait_value=rank[w.wait_value]))
                        changed = True
                    else:
                        new_wait.append(w)
                new_upd = list(si.on_update)
                if is_mm and c not in keepset:
                    kept_upd = [u for u in new_upd
                                if not (u.id == mm_sem and u.update_mode == "sem-inc")]
                    if len(kept_upd) != len(new_upd):
                        new_upd = kept_upd
                        changed = True
                if changed:
                    ins.sync_info = mybir.SyncInfo(on_wait=new_wait, on_update=new_upd)
    assert c == n_mm


_NC_CACHE = {}


def _get_nc(t_steps, h, w):
    key = (t_steps, h, w)
    if key not in _NC_CACHE:
        nc = build_nc(t_steps, h, w)
        _split_matmul_waits(nc)   # HW compile path only; CoreSim can't run these
        _thin_matmul_incs(nc)
        _NC_CACHE[key] = nc
    return _NC_CACHE[key]


def kernel(x, conv_w, conv_b):
    x = np.asarray(x, dtype=np.float32)
    conv_w = np.ascontiguousarray(np.asarray(conv_w), dtype=np.float32)
    conv_b = np.ascontiguousarray(np.asarray(conv_b), dtype=np.float32)
    # bias is a per-step additive constant into f: fold it into x on host
    x = np.ascontiguousarray(x + conv_b[None, None, :, None, None])
    b, t_steps, c, h, w = x.shape
    nc = _get_nc(t_steps, h, w)
    in_maps = [
        {"x": x[i], "conv_w": conv_w, "conv_b": conv_b} for i in range(b)
    ]
    res = run_bass_kernel_spmd(nc, in_maps, core_ids=list(range(b)))
    return np.stack([res.results[i]["y"] for i in range(b)], axis=0)


if __name__ == "__main__":
    nc = build_nc()
    print("built ok")

